# revision 20
# baseline (speedup 1.0000x reference)
"""CLUB loss kernel for 8x TRN2 NeuronCores.

Math: per sample b (L=512 positions, D=64 dims):
  mu     = MLP_mu(x);  logvar = tanh(MLP_lv(x));  iv = exp(-logvar)
  loss = mean_{b,l} sum_d (positive - negative)
       = -0.5/(B*L) * sum_{b,d,l} [ (y^2 - Ey2) - mu*yd2 ] * iv
with yd2 = 2*(y - Ey), Ey/Ey2 the per-(b,d) moments of y over positions.

Host precompute (same order of work as the packing transposes):
  r = (y^2 - Ey2) - b2mu*yd2  and  yd2, shipped bf16 stacked by L-half
  onto 128 partitions. The device computes only u = mu_nb*yd2,
  m = r - u, and sum(m*iv).

Layout trick: L is split in halves; half c lives on partitions
[64c, 64c+64) everywhere (y-side data, layer-2 PSUM outputs, iv), so
tanh/exp/final-accumulate each run ONCE on [128,256] instead of twice
on [64,256] (DVE/ACT cost is per-column, not per-partition).

b1 is folded into the layer-1 matmul via an appended ones-row on the
xb chunk (contraction K=65). A chain of 1x1 dummy matmuls before the
first real matmul keeps the PE busy through the input-DMA wait so the
PE p-state ramp (0.65 -> 1.2 -> 2.4 GHz after 3us continuous) is done
by the time real work arrives.

Sharding: data-parallel over batch B=8, one sample per core; host does
the tiny final combine.

Precision: matmul operands bf16; r/yd2/iv/u/m bf16 with f32
accumulation. Measured rel err vs fp32 reference ~1.5e-3, budget 2e-2.
"""

import sys

if "/opt/trn_rl_repo" not in sys.path:
    sys.path.insert(0, "/opt/trn_rl_repo")

import numpy as np

B, L = 8, 512
XD, YD, H = 192, 64, 128
NCORES = 8
HC = L // 2
NWARM = 12

_CACHE: dict = {}


def build_nc(debug: bool = False):
    import concourse.bass as bass
    import concourse.bacc as bacc
    import concourse.tile as tile
    from concourse import mybir
    from concourse.tile import add_dep_helper

    f32 = mybir.dt.float32
    bf16 = mybir.dt.bfloat16
    AF = mybir.ActivationFunctionType
    OP = mybir.AluOpType

    nc = bacc.Bacc("TRN2", target_bir_lowering=False, debug=debug)

    wiA_d = nc.dram_tensor("wiA", [128, 512], bf16, kind="ExternalInput")
    wiB_d = nc.dram_tensor("wiB", [128, 128], bf16, kind="ExternalInput")
    xa_d = nc.dram_tensor("xa", [128, L], bf16, kind="ExternalInput")
    xb_d = nc.dram_tensor("xb", [65, L], bf16, kind="ExternalInput")
    yq_d = nc.dram_tensor("yq", [128, L], bf16, kind="ExternalInput")
    aux_d = nc.dram_tensor("aux", [128, 1], f32, kind="ExternalInput")
    acc_d = nc.dram_tensor("acc", [2, 1], f32, kind="ExternalOutput")

    with tile.TileContext(nc) as tc:
        with (
            tc.tile_pool(name="sb", bufs=1) as sb,
            tc.tile_pool(name="ps", bufs=1, space=bass.MemorySpace.PSUM) as ps,
            tc.tile_pool(name="hps", bufs=3, space=bass.MemorySpace.PSUM) as hps,
            tc.tile_pool(name="wpool", bufs=1, space=bass.MemorySpace.PSUM) as wpool,
        ):
            # PE warmup fodder (tiny, written before any DMA lands)
            wu = sb.tile([1, 256], bf16, tag="wu")
            nc.gpsimd.memset(wu, 1.0)

            # input DMAs. HWDGE rings: scalar={wiA,wiB}, sync={xa,yq};
            # SWDGE (gpsimd): xb first (gates the b-part matmuls), aux.
            wiAt = sb.tile([128, 512], bf16, tag="wiAt")
            nc.scalar.dma_start(out=wiAt, in_=wiA_d[:, :])
            xat = sb.tile([128, L], bf16, tag="xat")
            nc.sync.dma_start(out=xat, in_=xa_d[:, :])
            xbt = sb.tile([128, L], bf16, tag="xbt")
            nc.sync.dma_start(out=xbt[0:65, :], in_=xb_d[:, :])
            wiBt = sb.tile([128, 128], bf16, tag="wiBt")
            nc.scalar.dma_start(out=wiBt, in_=wiB_d[:, :])
            yqt = sb.tile([128, L], bf16, tag="yqt")
            nc.sync.dma_start(out=yqt, in_=yq_d[:, :])
            auxt = sb.tile([128, 1], f32, tag="auxt")
            nc.gpsimd.dma_start(out=auxt, in_=aux_d[:, :])

            # PE p-state warmup: serial 256-col matmuls (WAW-chained)
            wps = wpool.tile([1, 256], f32, tag="wps")
            warm = []
            for i in range(NWARM):
                warm.append(
                    nc.tensor.matmul(
                        wps, wu[:, 0:1], wu[:, :], start=True, stop=True
                    )
                )

            w1lvT_a = wiAt[:, 0:128]
            w1muT_a = wiAt[:, 128:256]
            w1lvT_b = wiAt[0:65, 256:384]
            w1muT_b = wiAt[0:65, 384:512]
            w2lvT = wiBt[:, 0:64]
            w2muT = wiBt[:, 64:128]
            xb65 = xbt[0:65, :]
            b2lv_s = auxt[:, 0:1]

            acc2 = sb.tile([128, 2], f32, tag="acc2")
            h_lv_s = sb.tile([128, L], bf16, tag="hlvs")
            h_mu_s = sb.tile([128, L], bf16, tag="hmus")
            # one full 2KB PSUM bank per layer-2 output: matmul start=True
            # zeroes the whole bank, so tiles must not share banks
            z0 = ps.tile([64, 2 * HC], f32, tag="z0")
            z1b = ps.tile([128, 2 * HC], f32, tag="z1")
            mu0 = ps.tile([64, 2 * HC], f32, tag="mu0")
            mu1b = ps.tile([128, 2 * HC], f32, tag="mu1")
            zps = [z0[:, 0:HC], z1b[64:128, 0:HC]]
            mups = [mu0[:, 0:HC], mu1b[64:128, 0:HC]]
            u_s = sb.tile([128, HC], bf16, tag="u_s")
            m_s = sb.tile([128, HC], bf16, tag="m_s")
            iv_s = sb.tile([128, HC], bf16, tag="iv_s")
            t1 = sb.tile([128, HC], f32, tag="t1")
            scr = sb.tile([128, HC], bf16, tag="scr")

            # Ops are CREATED in a topological order that matches the
            # desired per-engine streams exactly. FILL(n) = n dummy 1x1
            # matmuls keeping the PE busy across a predicted sem wait so
            # the p-state ramp (full 2.4 GHz after 3us continuous) holds.
            #   PE : amu0 alv0 F bmu0 blv0 F w2mu0 alv1 blv1 amu1 bmu1
            #        w2lv0 F w2lv1 F w2mu1
            #   ACT: relu_lv0 relu_lv1 tanh0 tanh1 exp
            #   DVE: relu_mu0 u0 relu_mu1 m0 u1 m1 final
            c0, c1 = slice(0, HC), slice(HC, L)
            d0, d1 = slice(0, 64), slice(64, 128)

            h_mu0 = hps.tile([128, HC], f32, tag="h")
            h_lv0 = hps.tile([128, HC], f32, tag="h")
            mm_amu0 = nc.tensor.matmul(h_mu0, w1muT_a, xat[:, c0], start=True, stop=False)
            mm_alv0 = nc.tensor.matmul(h_lv0, w1lvT_a, xat[:, c0], start=True, stop=False)
            mm_bmu0 = nc.tensor.matmul(h_mu0, w1muT_b, xb65[:, c0], start=False, stop=True)
            relu_mu0 = nc.scalar.activation(
                out=h_mu_s[:, c0], in_=h_mu0, func=AF.Relu, scale=1.0
            )
            mm_blv0 = nc.tensor.matmul(h_lv0, w1lvT_b, xb65[:, c0], start=False, stop=True)
            relu_lv0 = nc.scalar.activation(
                out=h_lv_s[:, c0], in_=h_lv0, func=AF.Relu, scale=1.0
            )
            h_lv1 = hps.tile([128, HC], f32, tag="h")
            mm_alv1 = nc.tensor.matmul(h_lv1, w1lvT_a, xat[:, c1], start=True, stop=False)
            mm_blv1 = nc.tensor.matmul(h_lv1, w1lvT_b, xb65[:, c1], start=False, stop=True)
            relu_lv1 = nc.scalar.activation(
                out=h_lv_s[:, c1], in_=h_lv1, func=AF.Relu, scale=1.0
            )
            mm_w2mu0 = nc.tensor.matmul(
                mups[0], w2muT, h_mu_s[:, c0], start=True, stop=True
            )
            dve_u0 = nc.vector.scalar_tensor_tensor(
                out=u_s[d0, :], in0=mups[0], scalar=1.0,
                in1=yqt[d0, HC:L], op0=OP.mult, op1=OP.mult,
            )
            h_mu1 = hps.tile([128, HC], f32, tag="h")
            mm_amu1 = nc.tensor.matmul(h_mu1, w1muT_a, xat[:, c1], start=True, stop=False)
            mm_bmu1 = nc.tensor.matmul(h_mu1, w1muT_b, xb65[:, c1], start=False, stop=True)
            relu_mu1 = nc.vector.tensor_scalar_max(
                out=h_mu_s[:, c1], in0=h_mu1, scalar1=0.0
            )
            mm_w2lv0 = nc.tensor.matmul(
                zps[0], w2lvT, h_lv_s[:, c0], start=True, stop=True
            )
            tanh0 = nc.scalar.activation(
                out=t1[d0, :], in_=zps[0], func=AF.Tanh,
                bias=b2lv_s[d0, :], scale=1.0,
            )
            mm_w2lv1 = nc.tensor.matmul(
                zps[1], w2lvT, h_lv_s[:, c1], start=True, stop=True
            )
            tanh1 = nc.scalar.activation(
                out=t1[d1, :], in_=zps[1], func=AF.Tanh,
                bias=b2lv_s[d1, :], scale=1.0,
            )
            mm_w2mu1 = nc.tensor.matmul(
                mups[1], w2muT, h_mu_s[:, c1], start=True, stop=True
            )
            dve_u1 = nc.vector.scalar_tensor_tensor(
                out=u_s[d1, :], in0=mups[1], scalar=1.0,
                in1=yqt[d1, HC:L], op0=OP.mult, op1=OP.mult,
            )
            exp_s = nc.scalar.activation(out=iv_s, in_=t1, func=AF.Exp, scale=-1.0)
            # loss core = sum(r*iv) - sum(u*iv): two fused reduces
            ttrA = nc.vector.scalar_tensor_tensor(
                out=scr, in0=yqt[:, 0:HC], scalar=1.0, in1=iv_s,
                op0=OP.mult, op1=OP.mult, accum_out=acc2[:, 0:1],
            )
            ttrB = nc.vector.scalar_tensor_tensor(
                out=m_s, in0=u_s, scalar=1.0, in1=iv_s,
                op0=OP.mult, op1=OP.mult, accum_out=acc2[:, 1:2],
            )

            # collapse [128,1] accumulator to [1,1] on the PE; a wide
            # store walks partitions serially (~50ns each)
            ones = sb.tile([128, 1], f32, tag="ones")
            nc.gpsimd.memset(ones, 1.0)
            acc_ps = hps.tile([2, 1], f32, tag="h")
            mm_acc = nc.tensor.matmul(acc_ps, acc2, ones, start=True, stop=True)
            acc_sb = sb.tile([2, 1], f32, tag="accsb")
            nc.vector.tensor_copy(acc_sb, acc_ps)
            nc.sync.dma_start(out=acc_d[:, :], in_=acc_sb, single_packet=True)

            pe_order = warm + [
                mm_amu0, mm_alv0, mm_bmu0, mm_blv0, mm_alv1, mm_blv1,
                mm_w2mu0, mm_amu1, mm_bmu1,
                mm_w2lv0, mm_w2lv1, mm_w2mu1, mm_acc,
            ]
            act_order = [relu_mu0, relu_lv0, relu_lv1, tanh0, tanh1, exp_s]
            dve_order = [dve_u0, relu_mu1, dve_u1, ttrA, ttrB]
            for order in (pe_order, act_order, dve_order):
                for a, b in zip(order[1:], order[:-1]):
                    add_dep_helper(a.ins, b.ins, sync=False, reason="stream-order")

    nc.compile()
    return nc


def pack_inputs(inputs: dict) -> list[dict]:
    import ml_dtypes

    bf = ml_dtypes.bfloat16
    x = np.asarray(inputs["x_samples"], dtype=np.float32)
    y = np.ascontiguousarray(np.asarray(inputs["y_samples"], dtype=np.float32))
    mu_W1 = np.asarray(inputs["mu_W1"], dtype=np.float32)
    mu_b1 = np.asarray(inputs["mu_b1"], dtype=np.float32)
    mu_W2 = np.asarray(inputs["mu_W2"], dtype=np.float32)
    mu_b2 = np.asarray(inputs["mu_b2"], dtype=np.float32)
    lv_W1 = np.asarray(inputs["lv_W1"], dtype=np.float32)
    lv_b1 = np.asarray(inputs["lv_b1"], dtype=np.float32)
    lv_W2 = np.asarray(inputs["lv_W2"], dtype=np.float32)
    lv_b2 = np.asarray(inputs["lv_b2"], dtype=np.float32)

    # wiA: layer-1 weights; b-parts carry b1 as a 65th contraction row
    wiA = np.zeros((128, 512), bf)
    w1muT = mu_W1.T  # [192, 128]
    w1lvT = lv_W1.T
    wiA[:, 0:128] = w1lvT[0:128].astype(bf)
    wiA[:, 128:256] = w1muT[0:128].astype(bf)
    wiA[0:64, 256:384] = w1lvT[128:192].astype(bf)
    wiA[64, 256:384] = lv_b1.astype(bf)
    wiA[0:64, 384:512] = w1muT[128:192].astype(bf)
    wiA[64, 384:512] = mu_b1.astype(bf)
    wiB = np.zeros((128, 128), bf)
    wiB[:, 0:64] = lv_W2.T.astype(bf)
    wiB[:, 64:128] = mu_W2.T.astype(bf)
    # aux: b2lv replicated onto both half-stacks
    aux = np.empty((128, 1), np.float32)
    aux[0:64, 0] = lv_b2
    aux[64:128, 0] = lv_b2

    xb16 = x.astype(bf)
    # y-side precompute: r = (y^2 - Ey2) - b2mu*yd2, yd2 = 2*(y - Ey)
    ey = y.mean(axis=2, keepdims=True)  # [B, 64, 1]
    ey2 = (y * y).mean(axis=2, keepdims=True)
    yd2 = 2.0 * (y - ey)
    r = (y * y - ey2) - mu_b2[None, :, None] * yd2

    in_maps = []
    for b in range(NCORES):
        xb = np.empty((65, L), bf)
        xb[0:64] = xb16[b, 128:192]
        xb[64] = np.ones((L,), bf)
        # yq stacked: partitions [64c,64c+64) hold L-half c; cols 0:HC = r,
        # cols HC:L = yd2
        yq = np.empty((128, L), bf)
        for c in range(2):
            cs = slice(c * HC, (c + 1) * HC)
            yq[c * 64 : c * 64 + 64, 0:HC] = r[b][:, cs].astype(bf)
            yq[c * 64 : c * 64 + 64, HC:L] = yd2[b][:, cs].astype(bf)
        in_maps.append(
            {
                "wiA": wiA,
                "wiB": wiB,
                "xa": np.ascontiguousarray(xb16[b, 0:128]),
                "xb": xb,
                "yq": yq,
                "aux": aux,
            }
        )
    return in_maps


def _combine(results) -> float:
    tot = 0.0
    for r in results:
        a = r["acc"].astype(np.float64)
        tot += a[0, 0] - a[1, 0]
    return tot


def kernel(**inputs) -> np.ndarray:
    from concourse.bass_utils import run_bass_kernel_spmd

    if "nc" not in _CACHE:
        _CACHE["nc"] = build_nc(debug=False)
    nc = _CACHE["nc"]

    in_maps = pack_inputs(inputs)
    res = run_bass_kernel_spmd(nc, in_maps, core_ids=list(range(NCORES)))
    loss = -0.5 * _combine(res.results) / (B * L)
    return np.array(loss, dtype=np.float32)


# revision 21
# speedup vs baseline: 1.0245x; 1.0245x over previous
"""CLUB loss kernel for 8x TRN2 NeuronCores.

Math: per sample b (L=512 positions, D=64 dims):
  mu     = MLP_mu(x);  logvar = tanh(MLP_lv(x));  iv = exp(-logvar)
  loss = mean_{b,l} sum_d (positive - negative)
       = -0.5/(B*L) * sum_{b,d,l} [ (y^2 - Ey2) - mu*yd2 ] * iv
with yd2 = 2*(y - Ey), Ey/Ey2 the per-(b,d) moments of y over positions.

Host precompute (same order of work as the packing transposes):
  r = (y^2 - Ey2) - b2mu*yd2  and  yd2, shipped bf16 stacked by L-half
  onto 128 partitions. The device computes only u = mu_nb*yd2,
  m = r - u, and sum(m*iv).

Layout trick: L is split in halves; half c lives on partitions
[64c, 64c+64) everywhere (y-side data, layer-2 PSUM outputs, iv), so
tanh/exp/final-accumulate each run ONCE on [128,256] instead of twice
on [64,256] (DVE/ACT cost is per-column, not per-partition).

b1 is folded into the layer-1 matmul via an appended ones-row on the
xb chunk (contraction K=65). A chain of 1x1 dummy matmuls before the
first real matmul keeps the PE busy through the input-DMA wait so the
PE p-state ramp (0.65 -> 1.2 -> 2.4 GHz after 3us continuous) is done
by the time real work arrives.

Sharding: data-parallel over batch B=8, one sample per core; host does
the tiny final combine.

Precision: matmul operands bf16; r/yd2/iv/u/m bf16 with f32
accumulation. Measured rel err vs fp32 reference ~1.5e-3, budget 2e-2.
"""

import sys

if "/opt/trn_rl_repo" not in sys.path:
    sys.path.insert(0, "/opt/trn_rl_repo")

import numpy as np

B, L = 8, 512
XD, YD, H = 192, 64, 128
NCORES = 8
HC = L // 2
NWARM = 12

_CACHE: dict = {}


def build_nc(debug: bool = False):
    import concourse.bass as bass
    import concourse.bacc as bacc
    import concourse.tile as tile
    from concourse import mybir
    from concourse.tile import add_dep_helper

    f32 = mybir.dt.float32
    bf16 = mybir.dt.bfloat16
    AF = mybir.ActivationFunctionType
    OP = mybir.AluOpType

    nc = bacc.Bacc("TRN2", target_bir_lowering=False, debug=debug)

    wiA_d = nc.dram_tensor("wiA", [128, 512], bf16, kind="ExternalInput")
    wiB_d = nc.dram_tensor("wiB", [128, 128], bf16, kind="ExternalInput")
    xa_d = nc.dram_tensor("xa", [128, L], bf16, kind="ExternalInput")
    xb_d = nc.dram_tensor("xb", [65, L], bf16, kind="ExternalInput")
    yq_d = nc.dram_tensor("yq", [128, L], bf16, kind="ExternalInput")
    aux_d = nc.dram_tensor("aux", [128, 1], f32, kind="ExternalInput")
    acc_d = nc.dram_tensor("acc", [2, 1], f32, kind="ExternalOutput")

    with tile.TileContext(nc) as tc:
        with (
            tc.tile_pool(name="sb", bufs=1) as sb,
            tc.tile_pool(name="ps", bufs=1, space=bass.MemorySpace.PSUM) as ps,
            tc.tile_pool(name="hps", bufs=3, space=bass.MemorySpace.PSUM) as hps,
            tc.tile_pool(name="wpool", bufs=1, space=bass.MemorySpace.PSUM) as wpool,
        ):
            # PE warmup fodder (tiny, written before any DMA lands)
            wu = sb.tile([1, 256], bf16, tag="wu")
            nc.gpsimd.memset(wu, 1.0)

            # input DMAs. HWDGE rings: scalar={wiA,wiB}, sync={xa,yq};
            # SWDGE (gpsimd): xb first (gates the b-part matmuls), aux.
            wiAt = sb.tile([128, 512], bf16, tag="wiAt")
            nc.scalar.dma_start(out=wiAt, in_=wiA_d[:, :])
            xat = sb.tile([128, L], bf16, tag="xat")
            nc.sync.dma_start(out=xat, in_=xa_d[:, :])
            xbt = sb.tile([128, L], bf16, tag="xbt")
            nc.sync.dma_start(out=xbt[0:65, :], in_=xb_d[:, :])
            wiBt = sb.tile([128, 128], bf16, tag="wiBt")
            nc.scalar.dma_start(out=wiBt, in_=wiB_d[:, :])
            yqt = sb.tile([128, L], bf16, tag="yqt")
            nc.sync.dma_start(out=yqt, in_=yq_d[:, :])
            auxt = sb.tile([128, 1], f32, tag="auxt")
            nc.gpsimd.dma_start(out=auxt, in_=aux_d[:, :])

            # PE p-state warmup: serial 256-col matmuls (WAW-chained)
            wps = wpool.tile([1, 256], f32, tag="wps")
            warm = []
            for i in range(NWARM):
                warm.append(
                    nc.tensor.matmul(
                        wps, wu[:, 0:1], wu[:, :], start=True, stop=True
                    )
                )

            w1lvT_a = wiAt[:, 0:128]
            w1muT_a = wiAt[:, 128:256]
            w1lvT_b = wiAt[0:65, 256:384]
            w1muT_b = wiAt[0:65, 384:512]
            w2lvT = wiBt[:, 0:64]
            w2muT = wiBt[:, 64:128]
            xb65 = xbt[0:65, :]
            b2lv_s = auxt[:, 0:1]

            acc2 = sb.tile([128, 2], f32, tag="acc2")
            h_lv_s = sb.tile([128, L], bf16, tag="hlvs")
            h_mu_s = sb.tile([128, L], bf16, tag="hmus")
            # one full 2KB PSUM bank per layer-2 output: matmul start=True
            # zeroes the whole bank, so tiles must not share banks
            z0 = ps.tile([64, 2 * HC], f32, tag="z0")
            z1b = ps.tile([128, 2 * HC], f32, tag="z1")
            mu0 = ps.tile([64, 2 * HC], f32, tag="mu0")
            mu1b = ps.tile([128, 2 * HC], f32, tag="mu1")
            zps = [z0[:, 0:HC], z1b[64:128, 0:HC]]
            mups = [mu0[:, 0:HC], mu1b[64:128, 0:HC]]
            u_s = sb.tile([128, HC], bf16, tag="u_s")
            m_s = sb.tile([128, HC], f32, tag="m_s")
            iv_s = sb.tile([128, HC], bf16, tag="iv_s")
            t1 = sb.tile([128, HC], f32, tag="t1")
            scr = sb.tile([128, HC], f32, tag="scr")

            # Ops are CREATED in a topological order that matches the
            # desired per-engine streams exactly. FILL(n) = n dummy 1x1
            # matmuls keeping the PE busy across a predicted sem wait so
            # the p-state ramp (full 2.4 GHz after 3us continuous) holds.
            #   PE : amu0 alv0 F bmu0 blv0 F w2mu0 alv1 blv1 amu1 bmu1
            #        w2lv0 F w2lv1 F w2mu1
            #   ACT: relu_lv0 relu_lv1 tanh0 tanh1 exp
            #   DVE: relu_mu0 u0 relu_mu1 m0 u1 m1 final
            c0, c1 = slice(0, HC), slice(HC, L)
            d0, d1 = slice(0, 64), slice(64, 128)

            h_mu0 = hps.tile([128, HC], f32, tag="h")
            h_lv0 = hps.tile([128, HC], f32, tag="h")
            mm_amu0 = nc.tensor.matmul(h_mu0, w1muT_a, xat[:, c0], start=True, stop=False)
            mm_alv0 = nc.tensor.matmul(h_lv0, w1lvT_a, xat[:, c0], start=True, stop=False)
            mm_bmu0 = nc.tensor.matmul(h_mu0, w1muT_b, xb65[:, c0], start=False, stop=True)
            relu_mu0 = nc.scalar.activation(
                out=h_mu_s[:, c0], in_=h_mu0, func=AF.Relu, scale=1.0
            )
            mm_blv0 = nc.tensor.matmul(h_lv0, w1lvT_b, xb65[:, c0], start=False, stop=True)
            relu_lv0 = nc.scalar.activation(
                out=h_lv_s[:, c0], in_=h_lv0, func=AF.Relu, scale=1.0
            )
            h_lv1 = hps.tile([128, HC], f32, tag="h")
            mm_alv1 = nc.tensor.matmul(h_lv1, w1lvT_a, xat[:, c1], start=True, stop=False)
            mm_blv1 = nc.tensor.matmul(h_lv1, w1lvT_b, xb65[:, c1], start=False, stop=True)
            relu_lv1 = nc.scalar.activation(
                out=h_lv_s[:, c1], in_=h_lv1, func=AF.Relu, scale=1.0
            )
            mm_w2mu0 = nc.tensor.matmul(
                mups[0], w2muT, h_mu_s[:, c0], start=True, stop=True
            )
            dve_u0 = nc.vector.scalar_tensor_tensor(
                out=u_s[d0, :], in0=mups[0], scalar=1.0,
                in1=yqt[d0, HC:L], op0=OP.mult, op1=OP.mult,
            )
            h_mu1 = hps.tile([128, HC], f32, tag="h")
            mm_amu1 = nc.tensor.matmul(h_mu1, w1muT_a, xat[:, c1], start=True, stop=False)
            mm_bmu1 = nc.tensor.matmul(h_mu1, w1muT_b, xb65[:, c1], start=False, stop=True)
            relu_mu1 = nc.vector.tensor_scalar_max(
                out=h_mu_s[:, c1], in0=h_mu1, scalar1=0.0
            )
            mm_w2lv0 = nc.tensor.matmul(
                zps[0], w2lvT, h_lv_s[:, c0], start=True, stop=True
            )
            tanh0 = nc.scalar.activation(
                out=t1[d0, :], in_=zps[0], func=AF.Tanh,
                bias=b2lv_s[d0, :], scale=1.0,
            )
            mm_w2lv1 = nc.tensor.matmul(
                zps[1], w2lvT, h_lv_s[:, c1], start=True, stop=True
            )
            tanh1 = nc.scalar.activation(
                out=t1[d1, :], in_=zps[1], func=AF.Tanh,
                bias=b2lv_s[d1, :], scale=1.0,
            )
            mm_w2mu1 = nc.tensor.matmul(
                mups[1], w2muT, h_mu_s[:, c1], start=True, stop=True
            )
            dve_u1 = nc.vector.scalar_tensor_tensor(
                out=u_s[d1, :], in0=mups[1], scalar=1.0,
                in1=yqt[d1, HC:L], op0=OP.mult, op1=OP.mult,
            )
            exp_s = nc.scalar.activation(out=iv_s, in_=t1, func=AF.Exp, scale=-1.0)
            # loss core = sum(r*iv) - sum(u*iv): two fused reduces
            ttrA = nc.vector.scalar_tensor_tensor(
                out=scr, in0=yqt[:, 0:HC], scalar=1.0, in1=iv_s,
                op0=OP.mult, op1=OP.mult, accum_out=acc2[:, 0:1],
            )
            ttrB = nc.vector.scalar_tensor_tensor(
                out=m_s, in0=u_s, scalar=1.0, in1=iv_s,
                op0=OP.mult, op1=OP.mult, accum_out=acc2[:, 1:2],
            )

            # collapse [128,1] accumulator to [1,1] on the PE; a wide
            # store walks partitions serially (~50ns each)
            ones = sb.tile([128, 1], f32, tag="ones")
            nc.gpsimd.memset(ones, 1.0)
            acc_ps = hps.tile([2, 1], f32, tag="h")
            mm_acc = nc.tensor.matmul(acc_ps, acc2, ones, start=True, stop=True)
            acc_sb = sb.tile([2, 1], f32, tag="accsb")
            nc.vector.tensor_copy(acc_sb, acc_ps)
            nc.sync.dma_start(out=acc_d[:, :], in_=acc_sb, single_packet=True)

            pe_order = warm + [
                mm_amu0, mm_alv0, mm_bmu0, mm_blv0, mm_alv1, mm_blv1,
                mm_w2mu0, mm_amu1, mm_bmu1,
                mm_w2lv0, mm_w2lv1, mm_w2mu1, mm_acc,
            ]
            act_order = [relu_mu0, relu_lv0, relu_lv1, tanh0, tanh1, exp_s]
            dve_order = [dve_u0, relu_mu1, dve_u1, ttrA, ttrB]
            for order in (pe_order, act_order, dve_order):
                for a, b in zip(order[1:], order[:-1]):
                    add_dep_helper(a.ins, b.ins, sync=False, reason="stream-order")

    nc.compile()
    return nc


def pack_inputs(inputs: dict) -> list[dict]:
    import ml_dtypes

    bf = ml_dtypes.bfloat16
    x = np.asarray(inputs["x_samples"], dtype=np.float32)
    y = np.ascontiguousarray(np.asarray(inputs["y_samples"], dtype=np.float32))
    mu_W1 = np.asarray(inputs["mu_W1"], dtype=np.float32)
    mu_b1 = np.asarray(inputs["mu_b1"], dtype=np.float32)
    mu_W2 = np.asarray(inputs["mu_W2"], dtype=np.float32)
    mu_b2 = np.asarray(inputs["mu_b2"], dtype=np.float32)
    lv_W1 = np.asarray(inputs["lv_W1"], dtype=np.float32)
    lv_b1 = np.asarray(inputs["lv_b1"], dtype=np.float32)
    lv_W2 = np.asarray(inputs["lv_W2"], dtype=np.float32)
    lv_b2 = np.asarray(inputs["lv_b2"], dtype=np.float32)

    # wiA: layer-1 weights; b-parts carry b1 as a 65th contraction row
    wiA = np.zeros((128, 512), bf)
    w1muT = mu_W1.T  # [192, 128]
    w1lvT = lv_W1.T
    wiA[:, 0:128] = w1lvT[0:128].astype(bf)
    wiA[:, 128:256] = w1muT[0:128].astype(bf)
    wiA[0:64, 256:384] = w1lvT[128:192].astype(bf)
    wiA[64, 256:384] = lv_b1.astype(bf)
    wiA[0:64, 384:512] = w1muT[128:192].astype(bf)
    wiA[64, 384:512] = mu_b1.astype(bf)
    wiB = np.zeros((128, 128), bf)
    wiB[:, 0:64] = lv_W2.T.astype(bf)
    wiB[:, 64:128] = mu_W2.T.astype(bf)
    # aux: b2lv replicated onto both half-stacks
    aux = np.empty((128, 1), np.float32)
    aux[0:64, 0] = lv_b2
    aux[64:128, 0] = lv_b2

    xb16 = x.astype(bf)
    # y-side precompute: r = (y^2 - Ey2) - b2mu*yd2, yd2 = 2*(y - Ey)
    ey = y.mean(axis=2, keepdims=True)  # [B, 64, 1]
    ey2 = (y * y).mean(axis=2, keepdims=True)
    yd2 = 2.0 * (y - ey)
    r = (y * y - ey2) - mu_b2[None, :, None] * yd2

    in_maps = []
    for b in range(NCORES):
        xb = np.empty((65, L), bf)
        xb[0:64] = xb16[b, 128:192]
        xb[64] = np.ones((L,), bf)
        # yq stacked: partitions [64c,64c+64) hold L-half c; cols 0:HC = r,
        # cols HC:L = yd2
        yq = np.empty((128, L), bf)
        for c in range(2):
            cs = slice(c * HC, (c + 1) * HC)
            yq[c * 64 : c * 64 + 64, 0:HC] = r[b][:, cs].astype(bf)
            yq[c * 64 : c * 64 + 64, HC:L] = yd2[b][:, cs].astype(bf)
        in_maps.append(
            {
                "wiA": wiA,
                "wiB": wiB,
                "xa": np.ascontiguousarray(xb16[b, 0:128]),
                "xb": xb,
                "yq": yq,
                "aux": aux,
            }
        )
    return in_maps


def _combine(results) -> float:
    tot = 0.0
    for r in results:
        a = r["acc"].astype(np.float64)
        tot += a[0, 0] - a[1, 0]
    return tot


def kernel(**inputs) -> np.ndarray:
    from concourse.bass_utils import run_bass_kernel_spmd

    if "nc" not in _CACHE:
        _CACHE["nc"] = build_nc(debug=False)
    nc = _CACHE["nc"]

    in_maps = pack_inputs(inputs)
    res = run_bass_kernel_spmd(nc, in_maps, core_ids=list(range(NCORES)))
    loss = -0.5 * _combine(res.results) / (B * L)
    return np.array(loss, dtype=np.float32)
